# revision 22
# baseline (speedup 1.0000x reference)
"""Trainium2 Bass kernel for EnhancedGraphTransformerLayer.

Layer: LN1 -> QKV proj -> per-node 8x8 head attention -> O proj -> residual
       -> LN2 -> FFN(512->2048->512, relu) -> residual.

Strategy (per NeuronCore, data-parallel over nodes, 8 cores):
- Chunks of 512 nodes (4 groups of 128); all projection/FFN matmuls use
  N=512 moving operands (weights stationary, activations feature-transposed).
- Per-node 8-head attention via "sub-group" packing: for each 16-node
  sub-group, a (64, 128) layout q[d, h*16+j] lets one K=64 matmul compute all
  128x128 head-pair scores; block mask zeroes cross-node terms after exp; an
  appended ones-column of V yields softmax denominators inside the AV matmul.
- The packed layouts need odd-head data (which the QKV projections place on
  partitions 64-127) moved to partitions 0-63.  One bulk SBUF->SBUF "parity
  swap" DMA per tensor per chunk provides it; the packing itself is then
  partition-preserving strided engine copies (no per-head DMAs).
- O projection contracts K=128 by pairing heads (h, h+4) on partition halves
  (64x64 PE transposes write both PSUM partition halves directly).
- FFN2 uses activation tiles as stationary so its output is in natural
  layout (no final transposes).
- Chunks are software-pipelined: QKV(c) | FFN(c-1) | attention(c) so the FFN
  matmul burst covers the swap/pack latency and the PE never idles long
  enough for HAM to re-throttle.
"""

import numpy as np
import ml_dtypes
from contextlib import ExitStack

E = 512
H = 8
D = 64
F = 2048
EPS = 1e-5
N_NODES = 65536
N_CORES = 8
GPC = 4  # groups (of 128 nodes) per chunk
BF = ml_dtypes.bfloat16


def build_nc(npc, has_qkv_bias=False, has_bo=False, has_c2f=False,
             has_b2=False):
    import concourse.bass as bass
    import concourse.mybir as mybir

    f32 = mybir.dt.float32
    bf16 = mybir.dt.bfloat16
    f8 = mybir.dt.float8e4

    nc = bass.Bass()
    ins = dict(
        x=nc.dram_tensor("x", (npc, E), f32, kind="ExternalInput").ap(),
        rwq=nc.dram_tensor("rwq", (E, E), bf16, kind="ExternalInput").ap(),
        rwk=nc.dram_tensor("rwk", (E, E), bf16, kind="ExternalInput").ap(),
        rwv=nc.dram_tensor("rwv", (E, E), bf16, kind="ExternalInput").ap(),
        rwo=nc.dram_tensor("rwo", (E, E), bf16, kind="ExternalInput").ap(),
        rw1=nc.dram_tensor("rw1", (E, F), bf16, kind="ExternalInput").ap(),
        w2t=nc.dram_tensor("w2t", (F, E), bf16, kind="ExternalInput").ap(),
        mask=nc.dram_tensor("mask", (128, 128), bf16, kind="ExternalInput").ap(),
        c2q=nc.dram_tensor("c2q", (E,), f32, kind="ExternalInput").ap(),
        c2k=nc.dram_tensor("c2k", (E,), f32, kind="ExternalInput").ap(),
        c2v=nc.dram_tensor("c2v", (E,), f32, kind="ExternalInput").ap(),
        bo=nc.dram_tensor("bo", (E,), bf16, kind="ExternalInput").ap(),
        c2f=nc.dram_tensor("c2f", (F,), f32, kind="ExternalInput").ap(),
        b2=nc.dram_tensor("b2", (E,), bf16, kind="ExternalInput").ap(),
    )
    out_ap = nc.dram_tensor("out", (npc, E), f32, kind="ExternalOutput").ap()
    build_body(nc, ins, out_ap, npc, has_qkv_bias=has_qkv_bias,
               has_bo=has_bo, has_c2f=has_c2f, has_b2=has_b2)
    return nc


def build_body(nc, ins, out_d, npc, has_qkv_bias=False, has_bo=False,
               has_c2f=False, has_b2=False):
    import concourse.bass as bass
    import concourse.mybir as mybir
    from concourse.tile import TileContext
    from concourse.masks import make_identity

    f32 = mybir.dt.float32
    bf16 = mybir.dt.bfloat16
    f8 = mybir.dt.float8e4
    DR = mybir.MatmulPerfMode.DoubleRow
    AL = mybir.AluOpType
    AF = mybir.ActivationFunctionType
    QS, WS = 128.0, 16.0  # fp8 weight pre-scales (q-side, others)

    n_groups = npc // 128
    n_chunks = n_groups // GPC
    NC = GPC * 128  # nodes per chunk

    x_d = ins["x"]
    rwq_d, rwk_d, rwv_d, rwo_d = ins["rwq"], ins["rwk"], ins["rwv"], ins["rwo"]
    rw1_d, w2t_d, mask_d = ins["rw1"], ins["w2t"], ins["mask"]
    c2q_d, c2k_d, c2v_d = ins["c2q"], ins["c2k"], ins["c2v"]
    bo_d, c2f_d, b2_d = ins["bo"], ins["c2f"], ins["b2"]

    with TileContext(nc) as tc, ExitStack() as ctx:
        wpool = ctx.enter_context(tc.tile_pool(name="w", bufs=1))
        pool = ctx.enter_context(tc.tile_pool(name="act", bufs=1))
        psum = ctx.enter_context(tc.tile_pool(name="ps", bufs=1, space="PSUM"))

        # ---- constants / weights ----
        rwq_sb = wpool.tile([128, 4, E], bf16, tag="rwq")
        rwk_sb = wpool.tile([128, 4, E], bf16, tag="rwk")
        rwv_sb = wpool.tile([128, 4, E], bf16, tag="rwv")
        rwo_sb = wpool.tile([64, 8, E], bf16, tag="rwo")
        nc.sync.dma_start(out=rwq_sb, in_=rwq_d.rearrange("(t p) e -> p t e", p=128))
        nc.sync.dma_start(out=rwk_sb, in_=rwk_d.rearrange("(t p) e -> p t e", p=128))
        nc.sync.dma_start(out=rwv_sb, in_=rwv_d.rearrange("(t p) e -> p t e", p=128))
        nc.sync.dma_start(out=rwo_sb, in_=rwo_d.rearrange("(h d) e -> d h e", d=64))
        rw1_sb = wpool.tile([128, 4, F], bf16, tag="rw1")
        nc.sync.dma_start(out=rw1_sb, in_=rw1_d.rearrange("(t p) f -> p t f", p=128))
        w2t_sb = wpool.tile([128, 16, E], bf16, tag="w2t")
        nc.sync.dma_start(out=w2t_sb, in_=w2t_d.rearrange("(t p) e -> p t e", p=128))
        mask_sb = wpool.tile([128, 128], bf16, tag="mask")
        nc.sync.dma_start(out=mask_sb, in_=mask_d)
        ident64 = wpool.tile([64, 64], bf16, tag="id64")
        make_identity(nc, ident64)
        ident128 = wpool.tile([128, 128], bf16, tag="id128")
        make_identity(nc, ident128)
        eps_sb = wpool.tile([128, 1], f32, tag="eps")
        nc.vector.memset(eps_sb, EPS)
        if has_qkv_bias:
            c2q_sb = wpool.tile([128, 4], f32, tag="c2q")
            c2k_sb = wpool.tile([128, 4], f32, tag="c2k")
            c2v_sb = wpool.tile([128, 4], f32, tag="c2v")
            nc.sync.dma_start(out=c2q_sb, in_=c2q_d.rearrange("(t p) -> p t", p=128))
            nc.sync.dma_start(out=c2k_sb, in_=c2k_d.rearrange("(t p) -> p t", p=128))
            nc.sync.dma_start(out=c2v_sb, in_=c2v_d.rearrange("(t p) -> p t", p=128))
        if has_bo:
            ones1_sb = wpool.tile([1, 128], bf16, tag="ones1")
            nc.vector.memset(ones1_sb, 1.0)
            bo_sb = wpool.tile([1, E], bf16, tag="bo")
            nc.sync.dma_start(out=bo_sb, in_=bo_d.rearrange("e -> 1 e"))
        if has_c2f:
            c2f_sb = wpool.tile([128, 16], f32, tag="c2f")
            nc.sync.dma_start(out=c2f_sb, in_=c2f_d.rearrange("(t p) -> p t", p=128))
        if has_b2:
            if not has_bo:
                ones1_sb = wpool.tile([1, 128], bf16, tag="ones1")
                nc.vector.memset(ones1_sb, 1.0)
            b2_sb = wpool.tile([1, E], bf16, tag="b2")
            nc.sync.dma_start(out=b2_sb, in_=b2_d.rearrange("e -> 1 e"))

        def ap3(tile_ap, off, dims):
            """Custom AP: tile's partition dim + given free [stride, count]s."""
            return bass.AP(tensor=tile_ap.tensor, offset=tile_ap.offset + off,
                           ap=[tile_ap.ap[0]] + [list(d) for d in dims])

        def bcast8x64(small):
            return bass.AP(tensor=small.tensor, offset=small.offset,
                           ap=[small.ap[0], [1, 8], [0, 64]])

        def layernorm_to_bf16(x_ap, zb, tagp):
            stat = pool.tile([128, 6], f32, tag="stat", bufs=4, name=tagp + "st")
            nc.vector.bn_stats(out=stat, in_=x_ap)
            mv = pool.tile([128, 2], f32, tag="mv", bufs=4, name=tagp + "mv")
            nc.vector.bn_aggr(out=mv, in_=stat)
            rs = pool.tile([128, 1], f32, tag="rs", bufs=4, name=tagp + "rs")
            nc.scalar.activation(out=rs, in_=mv[:, 1:2], func=AF.Sqrt,
                                 bias=eps_sb, scale=1.0)
            nc.vector.reciprocal(out=rs, in_=rs)
            nc.vector.tensor_scalar(out=zb, in0=x_ap, scalar1=mv[:, 0:1],
                                    scalar2=rs, op0=AL.subtract, op1=AL.mult)

        # per-chunk state carried between pipeline stages (keyed by c % 2)
        st = [dict(), dict()]

        def load_x(c):
            s = st[c % 2]
            xs = []
            for g in range(GPC):
                xt = pool.tile([128, E], f32, tag="x", bufs=8, name="x_sb")
                nc.sync.dma_start(
                    out=xt,
                    in_=x_d[c * NC + g * 128:c * NC + (g + 1) * 128, :])
                xs.append(xt)
            s["x"] = xs

        def ln_stats(xs, mv4, g):
            stat = pool.tile([128, 6], f32, tag="stat", bufs=2, name="stat")
            nc.vector.bn_stats(out=stat, in_=xs)
            nc.vector.bn_aggr(out=mv4[:, g, :], in_=stat)

        def ln_apply(xs, zbs, mv4):
            """Batched LN tail: one Sqrt activation (one ACT table) + scale."""
            rs4 = pool.tile([128, GPC], f32, tag="rs", bufs=2, name="rs4")
            nc.scalar.activation(out=rs4, in_=ap3(mv4, 1, [[2, GPC]]),
                                 func=AF.Sqrt, bias=eps_sb, scale=1.0)
            nc.vector.reciprocal(out=rs4, in_=rs4)
            for g in range(GPC):
                nc.vector.tensor_scalar(out=zbs[g], in0=xs[g],
                                        scalar1=mv4[:, g, 0:1],
                                        scalar2=rs4[:, g:g + 1],
                                        op0=AL.subtract, op1=AL.mult)

        def ln2_z(c):
            """LN2 scale part (DVE/ACT only)."""
            s = st[c % 2]
            x2 = s["x2"]
            zbs = [pool.tile([128, E], bf16, tag="zb", bufs=2, name="z2b")
                   for _ in range(GPC)]
            ln_apply([x2[:, g, :] for g in range(GPC)], zbs, s["mv2"])
            z2bT = pool.tile([128, 4, NC], bf16, tag="z2bT", bufs=2,
                             name="z2bT")
            s["z2bT"] = z2bT

            def z2t_emit(g):
                tp2 = psum.tile([128, 4, 128], bf16, tag="tp", bufs=2,
                                name="z2T_ps")
                for tau in range(4):
                    nc.tensor.transpose(tp2[:, tau, :],
                                        zbs[g][:, 128 * tau:128 * (tau + 1)],
                                        ident128[:, :])
                nc.scalar.activation(out=z2bT[:, :, 128 * g:128 * (g + 1)],
                                     in_=tp2, func=AF.Copy)
            return z2t_emit

        def ln1_qkv(c):
            """LN1, z-transposes, QKV matmuls, packed-layout build DMAs."""
            s = st[c % 2]
            xs = s["x"]
            zbT = pool.tile([128, 4, NC], bf16, tag="zbT", bufs=2, name="zbT")
            zbs = [pool.tile([128, E], bf16, tag="zb", bufs=2, name="zb")
                   for _ in range(GPC)]
            mv4 = pool.tile([128, GPC, 2], f32, tag="mv", bufs=2, name="mv4")
            for g in range(GPC):
                ln_stats(xs[g], mv4, g)
            ln_apply(xs, zbs, mv4)
            for g in range(GPC):
                tp = psum.tile([128, 4, 128], bf16, tag="tp", bufs=2,
                               name="zT_ps")
                for tau in range(4):
                    nc.tensor.transpose(tp[:, tau, :],
                                        zbs[g][:, 128 * tau:128 * (tau + 1)],
                                        ident128[:, :])
                nc.scalar.activation(out=zbT[:, :, 128 * g:128 * (g + 1)],
                                     in_=tp, func=AF.Copy)

            # QKV projections, feature-transposed outputs [e_out, node]
            qkv_sb = []
            for ti in range(3):
                tb = pool.tile([128, 4, NC], bf16, tag="qkv%d" % ti, bufs=1,
                               name="qkv%d" % ti)
                qkv_sb.append(tb)
            for tau in range(4):
                for ti in range(3):
                    rw_sb = (rwq_sb, rwk_sb, rwv_sb)[ti]
                    acc = psum.tile([128, NC], f32, tag="acc", bufs=2,
                                    name="qkv_ps")
                    for et in range(4):
                        nc.tensor.matmul(acc,
                                         rw_sb[:, et, 128 * tau:128 * (tau + 1)],
                                         zbT[:, et, :],
                                         start=(et == 0), stop=(et == 3))
                    otb = qkv_sb[ti][:, tau, :]
                    if has_qkv_bias:
                        cs = (c2q_sb, c2k_sb, c2v_sb)[ti]
                        if ti == 2:
                            nc.vector.tensor_scalar_add(out=otb, in0=acc,
                                                        scalar1=cs[:, tau:tau + 1])
                        else:
                            nc.scalar.activation(out=otb, in_=acc,
                                                 func=AF.Identity,
                                                 bias=cs[:, tau:tau + 1])
                    else:
                        if ti == 1:
                            nc.scalar.activation(out=otb, in_=acc, func=AF.Copy)
                        else:
                            nc.vector.tensor_copy(out=otb, in_=acc)

            # q: dual-parity tensor (2 bulk DMAs); score MM streams a
            # strided view of it as the moving operand (no packed copy).
            qbig = pool.tile([64, 2, 4, NC], bf16, tag="qbig", bufs=1,
                             name="qbig")
            nc.sync.dma_start(out=qbig[:, 0, :, :], in_=qkv_sb[0][0:64, :, :])
            nc.gpsimd.dma_start(out=qbig[:, 1, :, :],
                                in_=qkv_sb[0][64:128, :, :])
            s["qbig"] = qbig
            # k, v: physically packed [64 d, g, s, h*16+j] via ACT copies;
            # odd-head halves come from bulk parity-swap DMAs.
            swk = pool.tile([64, 4, NC], bf16, tag="swk", bufs=1, name="swk")
            swv = pool.tile([64, 4, NC], bf16, tag="swv", bufs=1, name="swv")
            nc.gpsimd.dma_start(out=swk, in_=qkv_sb[1][64:128, :, :])
            nc.gpsimd.dma_start(out=swv, in_=qkv_sb[2][64:128, :, :])
            kxt = pool.tile([64, GPC, 8, 128], bf16, tag="kx", bufs=1,
                            name="kx")
            vxt = pool.tile([64, GPC, 8, 128], bf16, tag="vx", bufs=1,
                            name="vx")
            s["kx"], s["vx"] = kxt, vxt
            for g in range(GPC):
                for ti, t, src_o in ((1, kxt, swk), (2, vxt, swv)):
                    o_ev = ap3(t, g * 1024, [[128, 8], [32, 4], [1, 16]])
                    i_ev = ap3(qkv_sb[ti][0:64, :, :], g * 128,
                               [[16, 8], [NC, 4], [1, 16]])
                    o_od = ap3(t, g * 1024 + 16, [[128, 8], [32, 4], [1, 16]])
                    i_od = ap3(src_o, g * 128, [[16, 8], [NC, 4], [1, 16]])
                    if ti == 1:
                        nc.scalar.activation(out=o_ev, in_=i_ev, func=AF.Copy)
                        nc.scalar.activation(out=o_od, in_=i_od, func=AF.Copy)
                    else:
                        nc.gpsimd.tensor_copy(out=o_ev, in_=i_ev)
                        nc.gpsimd.tensor_copy(out=o_od, in_=i_od)

        def frontB(c, ffn_parts=()):
            """Attention with previous chunk's FFN woven in."""
            s = st[c % 2]
            xs = s["x"]
            qbig, kx, vx = s["qbig"], s["kx"], s["vx"]
            x2 = pool.tile([128, GPC, E], f32, tag="x2", bufs=2, name="x2")
            mv4 = pool.tile([128, GPC, 2], f32, tag="mv2", bufs=2, name="mv4b")
            s["x2"], s["mv2"] = x2, mv4

            def emit_scores(g):
                asl = []
                for sg in range(8):
                    s2 = psum.tile([128, 128], f32, tag="s2", bufs=2,
                                   name="s2_ps")
                    nc.tensor.matmul(
                        s2, kx[:, g, sg, :],
                        ap3(qbig, g * 128 + sg * 16,
                            [[NC, 4], [4 * NC, 2], [1, 16]]),
                        start=True, stop=True)
                    e_sb = pool.tile([128, 128], bf16, tag="esb", bufs=4,
                                     name="e_sb")
                    nc.scalar.activation(out=e_sb, in_=s2, func=AF.Exp)
                    a_sb = pool.tile([128, 128], bf16, tag="asb", bufs=33,
                                     name="a_sb")
                    nc.vector.tensor_tensor(out=a_sb, in0=e_sb, in1=mask_sb,
                                            op=AL.mult)
                    asl.append(a_sb)
                return asl

            def emit_vt(g):
                vp = psum.tile([128, 8, 64], bf16, tag="tp", bufs=2,
                               name="vp_ps")
                for sg in range(8):
                    nc.tensor.transpose(vp[:, sg, :], vx[:, g, sg, :],
                                        ident64[:, :])
                vaug = pool.tile([128, 8, 66], bf16, tag="vaug", bufs=4,
                                 name="vaug")
                nc.vector.memset(vaug[:, :, 64:65], 1.0)
                nc.vector.tensor_copy(out=vaug[:, :, 0:64], in_=vp)
                return vaug

            def emit_av(g):
                outS = psum.tile([128, 8, 128], f32, tag="outs", bufs=1,
                                 name="outS")
                for sg in range(8):
                    nc.tensor.matmul(outS[:, sg, 0:65], asls[g][sg],
                                     vaugs[g][:, sg, 0:65],
                                     start=True, stop=True)
                oS = pool.tile([128, 8, 66], bf16, tag="oS", bufs=3, name="oS")
                nc.scalar.activation(
                    out=oS, in_=ap3(outS, 0, [[128, 8], [1, 66]]),
                    func=AF.Copy)
                recip = pool.tile([128, 8], f32, tag="recip", bufs=4,
                                  name="recip")
                nc.vector.reciprocal(out=recip,
                                     in_=ap3(oS, 64, [[66, 8]]))
                ogb = pool.tile([128, 8, 64], bf16, tag="ogb", bufs=4,
                                name="ogb")
                nc.vector.tensor_tensor(
                    out=ogb, in0=ap3(oS, 0, [[66, 8], [1, 64]]),
                    in1=bcast8x64(recip), op=AL.mult)
                return ogb

            def emit_pt(g):
                pp = psum.tile([64, 8, 128], bf16, tag="tp", bufs=2,
                               name="p_ps")
                for sg in range(8):
                    nc.tensor.transpose(pp[:, sg, :], ogbs[g][:, sg, :],
                                        ident128[:, :])
                p_sb = pool.tile([64, 8, 128], bf16, tag="p4sb", bufs=4,
                                 name="p_sb")
                nc.vector.tensor_copy(
                    out=ap3(p_sb, 0, [[16, 8], [128, 8], [1, 16]]),
                    in_=ap3(pp, 0, [[128, 8], [16, 8], [1, 16]]))
                return p_sb

            def emit_oproj(g):
                op_ps = psum.tile([128, E], f32, tag="acc", bufs=2,
                                  name="oproj_ps")
                for h in range(H):
                    nc.tensor.matmul(op_ps, p_sbs[g][:, h, :], rwo_sb[:, h, :],
                                     start=(h == 0),
                                     stop=(h == 7 and not has_bo))
                if has_bo:
                    nc.tensor.matmul(op_ps, ones1_sb, bo_sb,
                                     start=False, stop=True)
                nc.vector.tensor_add(out=x2[:, g, :], in0=xs[g], in1=op_ps)
                ln_stats(x2[:, g, :], mv4, g)

            # stagger scores/vT then AV/pT then oproj, weaving the previous
            # chunk's FFN matmuls between units: the warm N=512 stream keeps
            # HAM at full clock through the attention phase (transposes and
            # short matmuls alone never re-warm it)
            asls, vaugs, ogbs, p_sbs = [], [], [], []

            def attn_units():
                def u_sc(g):
                    return lambda: asls.append(emit_scores(g))

                def u_vt(g):
                    return lambda: vaugs.append(emit_vt(g))

                def u_av(g):
                    return lambda: ogbs.append(emit_av(g))

                def u_pt(g):
                    return lambda: p_sbs.append(emit_pt(g))

                def u_op(g):
                    return lambda: emit_oproj(g)
                units = [u_sc(0), u_sc(1), u_sc(2), u_sc(3), u_vt(0), u_vt(1),
                         u_vt(2), u_vt(3)]
                for g in range(GPC):
                    units.append(u_av(g))
                    units.append(u_pt(g))
                for g in range(GPC):
                    units.append(u_op(g))
                return units

            units = attn_units()
            nf = len(ffn_parts)
            na = len(units)
            fi = 0
            for i, u in enumerate(units):
                u()
                want = (i + 1) * nf // na
                while fi < want:
                    ffn_parts[fi]()
                    fi += 1
            while fi < nf:
                ffn_parts[fi]()
                fi += 1


        def back1_parts(c):
            """FFN first layer -> 16 per-ft emitter closures."""
            s = st[c % 2]
            z2bT = s["z2bT"]
            rT = pool.tile([128, 16, NC], bf16, tag="rt", bufs=1, name="rT")
            s["rT"] = rT

            def emit_ft(ft):
                u1 = psum.tile([128, NC], f32, tag="acc", bufs=2, name="u1_ps")
                for et in range(4):
                    nc.tensor.matmul(u1,
                                     rw1_sb[:, et, 128 * ft:128 * (ft + 1)],
                                     z2bT[:, et, :],
                                     start=(et == 0), stop=(et == 3))
                if has_c2f:
                    nc.vector.tensor_scalar(out=rT[:, ft, :], in0=u1,
                                            scalar1=c2f_sb[:, ft:ft + 1],
                                            scalar2=0.0, op0=AL.add, op1=AL.max)
                elif ft % 2 == 0:
                    nc.scalar.activation(out=rT[:, ft, :], in_=u1, func=AF.Relu)
                else:
                    nc.vector.tensor_scalar_max(out=rT[:, ft, :], in0=u1,
                                                scalar1=0.0)
            return [lambda ft=ft: emit_ft(ft) for ft in range(16)]

        def back2_parts(c):
            """FFN second layer + residual 2 + store -> 5 closures."""
            s = st[c % 2]
            out_sb = pool.tile([128, GPC, E], f32, tag="osb", bufs=2,
                               name="out_sb")

            def emit_g(g):
                x2, rT = s["x2"], s["rT"]
                u2 = psum.tile([128, E], f32, tag="acc", bufs=2, name="u2_ps")
                for ft in range(16):
                    nc.tensor.matmul(u2, rT[:, ft, 128 * g:128 * (g + 1)],
                                     w2t_sb[:, ft, :],
                                     start=(ft == 0),
                                     stop=(ft == 15 and not has_b2))
                if has_b2:
                    nc.tensor.matmul(u2, ones1_sb, b2_sb,
                                     start=False, stop=True)
                nc.vector.tensor_add(out=out_sb[:, g, :], in0=s["x2"][:, g, :],
                                     in1=u2)

            def emit_store():
                nc.sync.dma_start(
                    out=out_d[c * NC:(c + 1) * NC, :].rearrange(
                        "(g p) e -> p g e", p=128),
                    in_=out_sb)
            return [lambda g=g: emit_g(g) for g in range(GPC)] + [emit_store]

        load_x(0)
        ln1_qkv(0)
        for c in range(n_chunks):
            if c + 1 < n_chunks:
                load_x(c + 1)
            if c > 0:
                ffn_parts = back1_parts(c - 1) + back2_parts(c - 1)
            else:
                ffn_parts = []
            frontB(c, ffn_parts)
            z2t = ln2_z(c)
            for g in range(GPC):
                z2t(g)
            if c + 1 < n_chunks:
                ln1_qkv(c + 1)
        for p in back1_parts(n_chunks - 1) + back2_parts(n_chunks - 1):
            p()

    _fix_sync_waits(nc)


_DMA_LIKE = ("InstDMACopy", "InstDmaTransposeAnt", "InstDMATranspose",
             "InstKVWritebackAnt", "InstPagedWritebackAnt")


def _fix_sync_waits(nc):
    """walrus limits inline sync waits to 1 per instruction. Tile can
    emit more. Split the excess into standalone InstEventSemaphore
    wait-carriers inserted immediately before the overweight instruction
    on the same engine - semantically identical."""
    import concourse.mybir as mybir
    n = 0
    for f in nc.m.functions:
        for blk in f.blocks:
            insts = blk.instructions
            out = []
            dirty = False
            for inst in insts:
                si = inst.sync_info
                waits = list(si.on_wait) if (si and si.on_wait) else []
                limit = 1
                if len(waits) > limit:
                    ups = list(si.on_update) if (si and si.on_update) else []
                    up_ids = {u.id for u in ups}
                    waits.sort(key=lambda w: 0 if w.id in up_ids else 1)
                    keep, move = waits[:limit], waits[limit:]
                    for w in move:
                        n += 1
                        car = mybir.InstEventSemaphore(
                            name="WSPLIT-%d" % n, ins=[], outs=[])
                        car.engine = inst.engine
                        car.sync_info = mybir.SyncInfo(on_wait=[w],
                                                       on_update=[])
                        out.append(car)
                    inst.sync_info = mybir.SyncInfo(on_wait=keep,
                                                   on_update=ups)
                    dirty = True
                out.append(inst)
            if dirty:
                blk.instructions = out
    return n


def _prep_weights(inputs):
    """Host-side weight folding. Returns dict of np arrays + flags."""
    f32 = np.float32
    g1 = np.asarray(inputs["g1"], f32)
    beta1 = np.asarray(inputs["beta1"], f32)
    g2 = np.asarray(inputs["g2"], f32)
    beta2 = np.asarray(inputs["beta2"], f32)
    Wq = np.asarray(inputs["Wq"], f32)
    Wk = np.asarray(inputs["Wk"], f32)
    Wv = np.asarray(inputs["Wv"], f32)
    Wo = np.asarray(inputs["Wo"], f32)
    W1 = np.asarray(inputs["W1"], f32)
    W2 = np.asarray(inputs["W2"], f32)
    scale = np.float32(1.0 / np.sqrt(D))

    rwq = (Wq.T * g1[:, None] * scale).astype(BF)
    rwk = (Wk.T * g1[:, None]).astype(BF)
    rwv = (Wv.T * g1[:, None]).astype(BF)
    rwo = Wo.T.astype(BF)
    rw1 = (W1.T * g2[:, None]).astype(BF)
    w2t = W2.T.astype(BF)

    c2q = ((Wq @ beta1 + np.asarray(inputs["bq"], f32)) * scale).astype(f32)
    c2k = (Wk @ beta1 + np.asarray(inputs["bk"], f32)).astype(f32)
    c2v = (Wv @ beta1 + np.asarray(inputs["bv"], f32)).astype(f32)
    bo = np.asarray(inputs["bo"], f32)
    c2f = (W1 @ beta2 + np.asarray(inputs["b1"], f32)).astype(f32)
    b2 = np.asarray(inputs["b2"], f32)

    mask = np.zeros((128, 128), f32)
    for i in range(16):
        for gg in range(8):
            for hh in range(8):
                mask[gg * 16 + i, hh * 16 + i] = 1.0

    return dict(
        rwq=rwq, rwk=rwk, rwv=rwv, rwo=rwo, rw1=rw1, w2t=w2t,
        mask=mask.astype(BF),
        c2q=c2q, c2k=c2k, c2v=c2v, bo=bo.astype(BF), c2f=c2f,
        b2=b2.astype(BF),
        has_qkv_bias=bool(np.any(c2q) or np.any(c2k) or np.any(c2v)),
        has_bo=bool(np.any(bo)), has_c2f=bool(np.any(c2f)),
        has_b2=bool(np.any(b2)),
    )


def kernel(**inputs):
    from concourse.bass_utils import run_bass_kernel_spmd

    x = np.asarray(inputs["x"], np.float32)
    n = x.shape[0]
    npc = n // N_CORES
    w = _prep_weights(inputs)

    nc = build_nc(npc, has_qkv_bias=w["has_qkv_bias"], has_bo=w["has_bo"],
                  has_c2f=w["has_c2f"], has_b2=w["has_b2"])

    shared = dict(rwq=w["rwq"], rwk=w["rwk"], rwv=w["rwv"], rwo=w["rwo"],
                  rw1=w["rw1"], w2t=w["w2t"], mask=w["mask"],
                  c2q=w["c2q"], c2k=w["c2k"], c2v=w["c2v"], bo=w["bo"],
                  c2f=w["c2f"], b2=w["b2"])
    in_maps = []
    for core in range(N_CORES):
        m = dict(shared)
        m["x"] = np.ascontiguousarray(x[core * npc:(core + 1) * npc])
        in_maps.append(m)

    res = run_bass_kernel_spmd(nc, in_maps, list(range(N_CORES)))
    out = np.concatenate([np.asarray(res.results[c]["out"])
                          for c in range(N_CORES)], axis=0)
    return out.astype(np.float32)
